# revision 11
# baseline (speedup 1.0000x reference)
"""DiagPooling (segment-reduce over square-image diagonals) on 8 NeuronCores.

Input  x: [8, 128, 512, 512] f32. Output: [8, 1, 513] f32 — per batch, the
mean over (channels, diagonal) of each diagonal offset in [-256, 256].

Sharding: batch b -> core b (data parallel, no communication).

Per-core pipeline (single-pass, no DRAM bounce):
1. Stream the 128 channels with partition p covering the flat range
   [2052*p, 2052*(p+1)) of each channel image (2052 = 4*513). Because flat
   (i, j) = 513*i + (j - i) indexes the stride-513 diagonal view
   P[q, r] = y_flat[513*q + r], partition p of the accumulator holds
   EXACTLY rows q = 4p..4p+3 of P — the channel sum lands pre-arranged for
   diagonal extraction, so no re-layout round-trip is needed. The host
   packs x as [128 ch, 128 p, 2064] with a 2064-element row pitch so every
   8208-byte read run is 64-byte aligned (the slow, HBM-latency-bound cores
   measurably benefit from aligned runs; row tails are zero-filled).
   The 128 per-channel 1 MiB loads alternate between the two HWDGE rings
   (sync + scalar) and accumulate on VectorE (2.3 us/channel, hidden under
   the ~2.5-3.3 us/channel DMA). Load tiles use a 64-byte-multiple pitch:
   a misaligned DVE operand costs ~20% (measured 2754 vs 2292 ns per add).
2. One masked multiply folds wanted(q, r) / (C * diag_len) into the
   accumulator; 3 DVE adds fold the 4 row-groups; two ones-vector matmuls
   (512 + 1 columns, PSUM-bank sized) give the 513 diagonal means.

Notes from HW measurement: the device-wide HBM ceiling is ~2.9 TB/s; each
run, a racy arbitration makes ~4 cores sustain ~421 GB/s (fabric line
rate) while the rest are stickily capped at ~320-330 GB/s regardless of
their own demand (self-pacing all cores to the fair share did NOT lift
the slow cores, so the cap is not demand-triggered). The kernel therefore
optimizes the slow-core critical path: minimal bytes, aligned runs,
shortest possible serial tail.
"""

import numpy as np

import concourse.bass as bass
import concourse.bacc as bacc
import concourse.mybir as mybir
from concourse import tile
from concourse.bass_utils import run_bass_kernel_spmd

B, C, H = 8, 128, 512
R = H + 1               # 513 distinct wanted diagonals
T = 4                   # P-view rows per partition
F = T * R               # 2052: accumulator free width (= flat elems/partition)
CH_ELEMS = H * H        # 262144 elements per (b, c) image
FP = 2064               # row pitch: 8256 B = 129*64 keeps runs/slots 64B-aligned
F32 = mybir.dt.float32


def _mask_qr() -> np.ndarray:
    """[512, 513] f64: wanted(q, r) / (C * diag_len)."""
    q = np.arange(H, dtype=np.int64)[:, None]
    r = np.arange(R, dtype=np.int64)[None, :]
    prefix = (r <= H // 2) & (q + r <= H - 1)            # diagonal o = r
    suffix = (r > H // 2) & (q + r >= H) & (q <= H - 2)  # o = r - 513
    mask = prefix | suffix
    o = np.where(r <= H // 2, r, r - R)
    denom = float(C) * (H - np.abs(o)).astype(np.float64)
    return mask.astype(np.float64) / denom


def _build_weights() -> np.ndarray:
    """[128, F] f32: the mask in the accumulator layout
    (row q = 4*p + t -> partition p, free column t*513 + r)."""
    return _mask_qr().reshape(128, T, R).reshape(128, F).astype(np.float32)


def _pack_x(xb: np.ndarray) -> np.ndarray:
    """[C*128*FP] f32: channel c, partition-row p at 64B-aligned pitch FP,
    holding the channel's flat range [2052*p, 2052*(p+1)) (zero tail)."""
    flat = np.ascontiguousarray(xb).reshape(C, CH_ELEMS)
    out = np.zeros((C, 128, FP), dtype=np.float32)
    src = np.lib.stride_tricks.as_strided(
        flat, shape=(C, 127, F), strides=(CH_ELEMS * 4, F * 4, 4)
    )
    out[:, :127, 0:F] = src
    out[:, 127, 0 : CH_ELEMS - 127 * F] = flat[:, 127 * F :]
    return out.reshape(-1)


def _build_program():
    nc = bacc.Bacc("TRN2", target_bir_lowering=False, debug=False, num_devices=B)
    xp = nc.dram_tensor("x", [C * 128 * FP], F32, kind="ExternalInput")
    wt = nc.dram_tensor("w", [128, F], F32, kind="ExternalInput")
    out_t = nc.dram_tensor("out", [1, R], F32, kind="ExternalOutput")

    NBUFS = 16

    with tile.TileContext(nc) as tc:
        with (
            tc.tile_pool(name="consts", bufs=1) as consts,
            tc.tile_pool(name="accp", bufs=1) as accp,
            tc.tile_pool(name="loadp", bufs=NBUFS) as loadp,
            tc.tile_pool(name="outp", bufs=1) as outp,
            tc.tile_pool(name="psum", bufs=2, space=bass.MemorySpace.PSUM) as psump,
        ):
            ones = consts.tile([128, 1], F32)
            nc.gpsimd.memset(ones[:], 1.0)
            w_tile = consts.tile([128, F], F32)

            # 1. channel stream in the diagonal-view layout, split over both
            # HWDGE rings; accumulate per channel on VectorE
            acc = accp.tile([128, F], F32)
            prev = None
            for c in range(C):
                t = loadp.tile([128, FP], F32)
                eng = nc.sync if c % 2 == 0 else nc.scalar
                eng.dma_start(
                    out=t[:, 0:F],
                    in_=bass.AP(xp, c * 128 * FP, [[FP, 128], [1, F]]),
                )
                if c == 112:
                    # mask weights ride both rings late (half each, keeping
                    # the rings' total byte loads equal so they finish
                    # together): they land well before the tail without
                    # adding to the start-up burst
                    nc.sync.dma_start(
                        out=w_tile[:, 0 : F // 2],
                        in_=bass.AP(wt, 0, [[F, 128], [1, F // 2]]),
                    )
                elif c == 113:
                    nc.scalar.dma_start(
                        out=w_tile[:, F // 2 : F],
                        in_=bass.AP(wt, F // 2, [[F, 128], [1, F - F // 2]]),
                    )
                if c == 0:
                    prev = t
                elif c == 1:
                    nc.vector.tensor_add(
                        out=acc[:], in0=prev[:, 0:F], in1=t[:, 0:F]
                    )
                else:
                    nc.vector.tensor_add(out=acc[:], in0=acc[:], in1=t[:, 0:F])

            # 2. mask, fold the 4 row-groups, column-sum via ones matmuls
            nc.vector.tensor_mul(out=acc[:], in0=acc[:], in1=w_tile[:])
            u = outp.tile([128, R], F32)
            nc.vector.tensor_add(out=u[:], in0=acc[:, 0:R], in1=acc[:, R : 2 * R])
            nc.vector.tensor_add(out=u[:], in0=u[:], in1=acc[:, 2 * R : 3 * R])
            nc.vector.tensor_add(out=u[:], in0=u[:], in1=acc[:, 3 * R : 4 * R])
            ps_a = psump.tile([1, 512], F32)
            ps_b = psump.tile([1, 1], F32)
            nc.tensor.matmul(ps_a[:], ones[:], u[:, 0:512], start=True, stop=True)
            nc.tensor.matmul(ps_b[:], ones[:], u[:, 512:513], start=True, stop=True)
            res = outp.tile([1, R], F32)
            nc.vector.tensor_copy(out=res[:, 0:512], in_=ps_a[:])
            nc.vector.tensor_copy(out=res[:, 512:513], in_=ps_b[:])
            nc.sync.dma_start(out=out_t.ap(), in_=res[:])

    nc.compile()
    return nc


_CACHE = {}


def kernel(x, _trace=False, _trace_cores=None) -> np.ndarray:
    x = np.asarray(x, dtype=np.float32)
    assert x.shape == (B, C, H, H), x.shape

    if "nc" not in _CACHE:
        _CACHE["nc"] = _build_program()
        _CACHE["w"] = _build_weights()
    nc = _CACHE["nc"]
    w = _CACHE["w"]

    in_maps = [{"x": _pack_x(x[b]), "w": w} for b in range(B)]
    result = run_bass_kernel_spmd(
        nc,
        in_maps,
        core_ids=list(range(B)),
        trace=_trace,
        trace_cores=_trace_cores,
    )
    _CACHE["last_result"] = result

    out = np.empty((B, 1, R), dtype=np.float32)
    for b in range(B):
        r = result.results[b]["out"].reshape(R)
        # column r -> offset o = r (r <= 256) / r - 513 (r >= 257);
        # output index n = o + 256
        out[b, 0, :] = np.concatenate([r[R - 256 :], r[: R - 256]])
    return out


# revision 13
# speedup vs baseline: 1.0415x; 1.0415x over previous
"""DiagPooling (segment-reduce over square-image diagonals) on 8 NeuronCores.

Input  x: [8, 128, 512, 512] f32. Output: [8, 1, 513] f32 — per batch, the
mean over (channels, diagonal) of each diagonal offset in [-256, 256].

Sharding: batch b -> core b (data parallel, no communication).

Per-core pipeline (single-pass, no DRAM bounce):
1. Stream the 128 channels with partition p covering the flat range
   [2052*p, 2052*(p+1)) of each channel image (2052 = 4*513). Because flat
   (i, j) = 513*i + (j - i) indexes the stride-513 diagonal view
   P[q, r] = y_flat[513*q + r], partition p of the accumulator holds
   EXACTLY rows q = 4p..4p+3 of P — the channel sum lands pre-arranged for
   diagonal extraction, so no re-layout round-trip is needed. The host
   packs x as [128 ch, 128 p, 2064] with a 2064-element row pitch so every
   8208-byte read run is 64-byte aligned (the slow, HBM-latency-bound cores
   measurably benefit from aligned runs; row tails are zero-filled).
   The 128 per-channel 1 MiB loads alternate between the two HWDGE rings
   (sync + scalar) and accumulate on VectorE (2.3 us/channel, hidden under
   the ~2.5-3.3 us/channel DMA). Load tiles use a 64-byte-multiple pitch:
   a misaligned DVE operand costs ~20% (measured 2754 vs 2292 ns per add).
2. One masked multiply folds wanted(q, r) / (C * diag_len) into the
   accumulator; 3 DVE adds fold the 4 row-groups; two ones-vector matmuls
   (512 + 1 columns, PSUM-bank sized) give the 513 diagonal means.

Notes from HW measurement: the device-wide HBM ceiling is ~2.9 TB/s; each
run, a racy arbitration makes ~4 cores sustain ~421 GB/s (fabric line
rate) while the rest are stickily capped at ~320-330 GB/s regardless of
their own demand (self-pacing all cores to the fair share did NOT lift
the slow cores, so the cap is not demand-triggered). The kernel therefore
optimizes the slow-core critical path: minimal bytes, aligned runs,
shortest possible serial tail.
"""

import numpy as np

import concourse.bass as bass
import concourse.bacc as bacc
import concourse.mybir as mybir
from concourse import tile
from concourse.bass_utils import run_bass_kernel_spmd

B, C, H = 8, 128, 512
R = H + 1               # 513 distinct wanted diagonals
T = 4                   # P-view rows per partition
F = T * R               # 2052: accumulator free width (= flat elems/partition)
CH_ELEMS = H * H        # 262144 elements per (b, c) image
FP = 2064               # row pitch: 8256 B = 129*64 keeps runs/slots 64B-aligned
PACE_N = 3104           # ACT pacer width -> ~2.90 us issue period (~362 GB/s)
F32 = mybir.dt.float32


def _mask_qr() -> np.ndarray:
    """[512, 513] f64: wanted(q, r) / (C * diag_len)."""
    q = np.arange(H, dtype=np.int64)[:, None]
    r = np.arange(R, dtype=np.int64)[None, :]
    prefix = (r <= H // 2) & (q + r <= H - 1)            # diagonal o = r
    suffix = (r > H // 2) & (q + r >= H) & (q <= H - 2)  # o = r - 513
    mask = prefix | suffix
    o = np.where(r <= H // 2, r, r - R)
    denom = float(C) * (H - np.abs(o)).astype(np.float64)
    return mask.astype(np.float64) / denom


def _build_weights() -> np.ndarray:
    """[128, F] f32: the mask in the accumulator layout
    (row q = 4*p + t -> partition p, free column t*513 + r)."""
    return _mask_qr().reshape(128, T, R).reshape(128, F).astype(np.float32)


def _pack_x(xb: np.ndarray) -> np.ndarray:
    """[C*128*FP] f32: channel c, partition-row p at 64B-aligned pitch FP,
    holding the channel's flat range [2052*p, 2052*(p+1)) (zero tail)."""
    flat = np.ascontiguousarray(xb).reshape(C, CH_ELEMS)
    out = np.zeros((C, 128, FP), dtype=np.float32)
    src = np.lib.stride_tricks.as_strided(
        flat, shape=(C, 127, F), strides=(CH_ELEMS * 4, F * 4, 4)
    )
    out[:, :127, 0:F] = src
    out[:, 127, 0 : CH_ELEMS - 127 * F] = flat[:, 127 * F :]
    return out.reshape(-1)


def _build_program():
    nc = bacc.Bacc("TRN2", target_bir_lowering=False, debug=False, num_devices=B)
    xp = nc.dram_tensor("x", [C * 128 * FP], F32, kind="ExternalInput")
    wt = nc.dram_tensor("w", [128, F], F32, kind="ExternalInput")
    out_t = nc.dram_tensor("out", [1, R], F32, kind="ExternalOutput")

    NBUFS = 16

    with tile.TileContext(nc) as tc:
        with (
            tc.tile_pool(name="consts", bufs=1) as consts,
            tc.tile_pool(name="accp", bufs=1) as accp,
            tc.tile_pool(name="loadp", bufs=NBUFS) as loadp,
            tc.tile_pool(name="outp", bufs=1) as outp,
            tc.tile_pool(name="psum", bufs=2, space=bass.MemorySpace.PSUM) as psump,
        ):
            ones = consts.tile([128, 1], F32)
            nc.gpsimd.memset(ones[:], 1.0)
            pace = consts.tile([128, PACE_N], F32)
            nc.gpsimd.memset(pace[:], 0.0)
            w_tile = consts.tile([128, F], F32)

            # 1. paced channel stream in the diagonal-view layout on the
            # scalar (ACT) HWDGE ring; accumulate per channel on VectorE.
            # The ACT queue is serial, so the calibrated ACTIVATE between
            # consecutive dma_starts meters each core to ~362 GB/s; with
            # every core polite and aligned, aggregate demand (2.90 TB/s)
            # stays under the ~3.06 TB/s device cap and no core gets
            # demoted to the sticky ~328 GB/s loser mode.
            acc = accp.tile([128, F], F32)
            prev = None
            for c in range(C):
                t = loadp.tile([128, FP], F32)
                nc.scalar.dma_start(
                    out=t[:, 0:F],
                    in_=bass.AP(xp, c * 128 * FP, [[FP, 128], [1, F]]),
                )
                if c == 113:
                    # mask weights ride the paced ring late: they land well
                    # before the tail without adding to the start-up burst
                    nc.scalar.dma_start(out=w_tile[:], in_=wt.ap())
                if c < C - 1:
                    nc.scalar.activation(
                        out=pace[:], in_=pace[:],
                        func=mybir.ActivationFunctionType.Copy,
                    )
                if c == 0:
                    prev = t
                elif c == 1:
                    nc.vector.tensor_add(
                        out=acc[:], in0=prev[:, 0:F], in1=t[:, 0:F]
                    )
                else:
                    nc.vector.tensor_add(out=acc[:], in0=acc[:], in1=t[:, 0:F])

            # 2. mask, fold the 4 row-groups, column-sum via ones matmuls
            nc.vector.tensor_mul(out=acc[:], in0=acc[:], in1=w_tile[:])
            u = outp.tile([128, R], F32)
            nc.vector.tensor_add(out=u[:], in0=acc[:, 0:R], in1=acc[:, R : 2 * R])
            nc.vector.tensor_add(out=u[:], in0=u[:], in1=acc[:, 2 * R : 3 * R])
            nc.vector.tensor_add(out=u[:], in0=u[:], in1=acc[:, 3 * R : 4 * R])
            ps_a = psump.tile([1, 512], F32)
            ps_b = psump.tile([1, 1], F32)
            nc.tensor.matmul(ps_a[:], ones[:], u[:, 0:512], start=True, stop=True)
            nc.tensor.matmul(ps_b[:], ones[:], u[:, 512:513], start=True, stop=True)
            res = outp.tile([1, R], F32)
            nc.vector.tensor_copy(out=res[:, 0:512], in_=ps_a[:])
            nc.vector.tensor_copy(out=res[:, 512:513], in_=ps_b[:])
            nc.sync.dma_start(out=out_t.ap(), in_=res[:])

    nc.compile()
    return nc


_CACHE = {}


def kernel(x, _trace=False, _trace_cores=None) -> np.ndarray:
    x = np.asarray(x, dtype=np.float32)
    assert x.shape == (B, C, H, H), x.shape

    if "nc" not in _CACHE:
        _CACHE["nc"] = _build_program()
        _CACHE["w"] = _build_weights()
    nc = _CACHE["nc"]
    w = _CACHE["w"]

    in_maps = [{"x": _pack_x(x[b]), "w": w} for b in range(B)]
    result = run_bass_kernel_spmd(
        nc,
        in_maps,
        core_ids=list(range(B)),
        trace=_trace,
        trace_cores=_trace_cores,
    )
    _CACHE["last_result"] = result

    out = np.empty((B, 1, R), dtype=np.float32)
    for b in range(B):
        r = result.results[b]["out"].reshape(R)
        # column r -> offset o = r (r <= 256) / r - 513 (r >= 257);
        # output index n = o + 256
        out[b, 0, :] = np.concatenate([r[R - 256 :], r[: R - 256]])
    return out
